# revision 4
# baseline (speedup 1.0000x reference)
"""2-layer GAT (GATConv x2, PyG-style with self-loops) on 8 Trainium2 NeuronCores.

Strategy (graph/data parallel, per sharding hint):
- Nodes are sharded across 8 cores (12500 each, padded to 12544). Each core
  computes the layer projections for its node shard, with the attention
  vectors folded into the projection weights on the host
  (W_aug = [W@Asrc | W | W@Adst]), and writes a per-node 256-byte table row
  [al_src(8) | feat | pad] to DRAM. Tables are AllGathered (Shared
  scratchpad) so every core holds all source-node rows (halo).
- Incident edges are partitioned by destination; self-loops are ordinary
  edges. Per 128-destination tile, edges sit in slot columns (dest on
  partitions). Slots are fetched with nc.gpsimd.dma_gather
  (InstDMAGatherAnt, single_packet=False), whose Q7 ucode generates
  descriptors ~15x faster than the generic indirect-DMA path (measured
  ~0.7 vs ~11 ns/row): row i of a call lands on partition i%128, column
  i//128 - exactly the slot grid. dma_gather indices are int16, so the
  100352-row table is split into four 25088-row quarters; each tile's slot
  columns are grouped by source-quarter (padded per quarter to the max
  per-partition count over cores, ~2.4x column inflation) and fetched by
  per-quarter calls of <=896 rows (SWDGE descriptor-ring limit ~1024).
  Filler slots point at a pad row whose al_src is -60000, so exp()
  contributes exactly 0. Attention softmax and the weighted aggregation run
  as wide strided DVE/ACT ops over the gathered block; max-subtraction is
  skipped (logits are O(1), exp is stable, exp(e-m)/sum == exp(e)/sum).
  Alpha is pre-normalized (p * 1/den) in fp16 for 2x DVE throughput.

Host side does only index/graph preprocessing (shard, degree-sort, slot
packing, quarter split) plus weight folding; all FLOPs on the N-dimension
run on device.
"""

import numpy as np

# Problem constants (hardcoded per spec)
N = 100000
E = 1600000
F_IN = 512
HID = 8
HEADS = 8
F_HID = HID * HEADS  # 64
NUM_CLASSES = 40
NEG_SLOPE = 0.2
CORES = 8
P = 128
ROW = 128           # table row stride in fp16 elements (256 B, dma_gather min)
NQ = 4              # int16 index range => 4 table quarters
BIG_NEG = -60000.0  # al_src of dummy/pad rows; exp(leaky(.)) == 0.0 exactly
MAXC = 7            # max slot columns per dma_gather call (<=896 descriptors)
NQUEUES = 4         # SWDGE queues; round-robin parallelizes Q7 desc-gen ~4.7x

_PROGRAM_CACHE = {}


def _split_shards(n, cores):
    base = n // cores
    rem = n % cores
    sizes = [base + (1 if r < rem else 0) for r in range(cores)]
    offs = np.concatenate([[0], np.cumsum(sizes)])
    return sizes, offs


def _preprocess_graph(edge_index, n=N, cores=CORES):
    """Shard dst nodes, degree-sort within shard, pack per-quarter edge slots.

    Returns per-core wrapped int16 index arrays plus the shared tile/chunk
    structure (identical across cores so the SPMD program is uniform).
    """
    loops = np.arange(n, dtype=np.int64)
    src = np.concatenate([edge_index[0].astype(np.int64), loops])
    dst = np.concatenate([edge_index[1].astype(np.int64), loops])

    sizes, offs = _split_shards(n, cores)
    shard_pad = int(np.ceil((max(sizes) + 1) / P) * P)  # 12544
    tiles = shard_pad // P
    qsz = shard_pad * cores // NQ  # 25088 rows per quarter (= 2 shards)

    deg = np.bincount(dst, minlength=n)
    node_core = np.searchsorted(offs[1:], np.arange(n), side="right")

    perms = []
    node_pos = np.zeros(n, dtype=np.int64)
    for r in range(cores):
        own = np.arange(offs[r], offs[r + 1])
        order = own[np.argsort(-deg[own], kind="stable")]
        perms.append(order)
        node_pos[order] = np.arange(len(order))

    table_row = node_core * shard_pad + node_pos

    # per-(tile, quarter) slot counts, uniform across cores
    Cq = np.zeros((tiles, NQ), dtype=np.int64)
    per_core = []
    for r in range(cores):
        m = (dst >= offs[r]) & (dst < offs[r + 1])
        e_src = src[m]
        e_dst = dst[m]
        lpos = node_pos[e_dst]
        rows = table_row[e_src]
        q = rows // qsz
        t = lpos // P
        p = lpos % P
        cnt = np.zeros((tiles, P, NQ), dtype=np.int64)
        np.add.at(cnt, (t, p, q), 1)
        Cq = np.maximum(Cq, cnt.max(axis=1))
        per_core.append((e_src, lpos, rows, q, t, p))
    Cq[:, 0] = np.maximum(Cq[:, 0], 1)  # room for the pad-dst anchor (row 0)

    # column layout per tile: [q0 block | q1 | q2 | q3]
    qoff = np.concatenate([np.zeros((tiles, 1), np.int64),
                           np.cumsum(Cq, axis=1)], axis=1)  # [tiles, 5]
    CT = qoff[:, NQ]          # total columns per tile
    toff = np.concatenate([[0], np.cumsum(CT)])  # tile column offsets
    S = int(toff[-1])

    DUMMY_LOCAL = sizes[0]  # pad row of the quarter's low shard (= 12500)

    idx_arrays = []
    for r in range(cores):
        e_src, lpos, rows, q, t, p = per_core[r]
        grid = np.full((P, S), DUMMY_LOCAL, dtype=np.int64)
        # slot within (tile, partition, quarter), via stable sort by (q,t,p)
        order = np.lexsort((p, t, q))
        qq, tt, pp, rr = q[order], t[order], p[order], rows[order]
        key = (qq * tiles + tt) * P + pp
        grp_start = np.searchsorted(key, key, side="left")
        slot = np.arange(len(key)) - grp_start
        col = toff[tt] + qoff[tt, qq] + slot
        grid[pp, col] = rr - qq * qsz
        # padded dst rows: anchor slot 0 (a q0 column) to row 0
        n_real = sizes[r]
        if shard_pad > n_real:
            pl = np.arange(n_real, shard_pad)
            grid[pl % P, toff[pl // P]] = 0
        idx_arrays.append(grid.astype(np.int16))

    # chunk list per tile: (q, col_start_in_tile, n_cols)
    chunks = []
    for t in range(tiles):
        ch = []
        for q in range(NQ):
            c0 = int(qoff[t, q])
            left = int(Cq[t, q])
            while left > 0:
                cl = min(MAXC, left)
                ch.append((q, c0, cl))
                c0 += cl
                left -= cl
        chunks.append(ch)

    # wrapped idx DRAM image [P, sum_over_chunks(cl*8)] int16 per core:
    # per chunk, flat list (c-major, p-minor) wrapped 16-way, replicated 8x.
    wtoff = []  # per tile, start column (in int16 units) of its idx block
    wpos = 0
    for t in range(tiles):
        wtoff.append(wpos)
        wpos += int(CT[t]) * 8
    WTOT = wpos
    idx_wrapped = []
    for r in range(cores):
        grid = idx_arrays[r]
        img = np.zeros((P, WTOT), dtype=np.int16)
        for t in range(tiles):
            pos = wtoff[t]
            for (q, c0, cl) in chunks[t]:
                cols = grid[:, toff[t] + c0: toff[t] + c0 + cl]  # [P, cl]
                flat = cols.T.reshape(-1)                        # c-major
                wr = flat.reshape(cl * 8, 16).T                  # [16, cl*8]
                img[:, pos:pos + cl * 8] = np.tile(wr, (8, 1))
                pos += cl * 8
        idx_wrapped.append(img)

    return {
        "idx": idx_wrapped,
        "perms": perms,
        "sizes": sizes,
        "offs": offs,
        "shard_pad": shard_pad,
        "tiles": tiles,
        "Cq": Cq,
        "CT": CT,
        "toff": toff,
        "chunks": chunks,
        "wtoff": wtoff,
        "WTOT": WTOT,
        "qsz": qsz,
        "pad_rows_start": [sizes[r] for r in range(cores)],
    }


def _build_program(meta, repeat=1):
    """Build the SPMD Bass program (identical across cores)."""
    from concourse import mybir, bacc
    import concourse.tile as tile
    from concourse.masks import make_identity

    dt = mybir.dt
    SH = meta["shard_pad"]
    TILES = meta["tiles"]
    NROWS = SH * CORES
    W1C = F_IN // P
    T1W = HEADS + F_HID  # 72 used fp16 of each 128-elem row
    T2W = 1 + NUM_CLASSES  # 41
    CT = meta["CT"]
    chunks = meta["chunks"]
    wtoff = meta["wtoff"]
    WTOT = meta["WTOT"]
    qsz = meta["qsz"]

    nc = bacc.Bacc("TRN2", target_bir_lowering=False, debug=False,
                   num_devices=CORES, num_swdge_queues=NQUEUES)
    xT = nc.dram_tensor("xT", [F_IN, SH], dt.float16, kind="ExternalInput")
    w1aug = nc.dram_tensor("w1aug", [F_IN, 80], dt.float16, kind="ExternalInput")
    w2aug = nc.dram_tensor("w2aug", [F_HID, 42], dt.float16, kind="ExternalInput")
    b1rep = nc.dram_tensor("b1rep", [P, F_HID], dt.float32, kind="ExternalInput")
    b2rep = nc.dram_tensor("b2rep", [P, NUM_CLASSES], dt.float32, kind="ExternalInput")
    idx_in = nc.dram_tensor("idx", [P, WTOT], dt.int16, kind="ExternalInput")
    out = nc.dram_tensor("out", [SH, NUM_CLASSES], dt.float32, kind="ExternalOutput")

    AF = mybir.ActivationFunctionType
    OP = mybir.AluOpType
    AX = mybir.AxisListType

    with tile.TileContext(nc) as tc:
        with (
            tc.tile_pool(name="const", bufs=1) as cpool,
            tc.tile_pool(name="resident", bufs=1) as rpool,
            tc.tile_pool(name="work", bufs=3) as wpool,
            tc.tile_pool(name="gbuf", bufs=3) as gpool,
            tc.tile_pool(name="ibuf", bufs=3) as ipool,
            tc.tile_pool(name="psum", bufs=2, space="PSUM") as ppool,
            tc.tile_pool(name="dram", bufs=1, space="DRAM") as dpool,
        ):
            # ---- constants / residents ----
            w1_t = cpool.tile([P, W1C * 80], dt.float16)
            for c in range(W1C):
                nc.sync.dma_start(out=w1_t[:, c * 80:(c + 1) * 80],
                                  in_=w1aug[c * P:(c + 1) * P, :])
            w2_t = cpool.tile([F_HID, 42], dt.float16)
            nc.sync.dma_start(out=w2_t[:], in_=w2aug[:, :])
            b1_t = cpool.tile([P, F_HID], dt.float32)
            nc.sync.dma_start(out=b1_t[:], in_=b1rep[:, :])
            b2_t = cpool.tile([P, NUM_CLASSES], dt.float32)
            nc.sync.dma_start(out=b2_t[:], in_=b2rep[:, :])
            ident = cpool.tile([P, P], dt.float32)
            make_identity(nc, ident[:])
            ald1 = rpool.tile([P, TILES * HEADS], dt.float32)
            ald2 = rpool.tile([P, TILES], dt.float32)

            npad = SH - meta["pad_rows_start"][0]

            qrr = [0]  # round-robin queue counter (shared across layers)

            def gather_tile(G, t, full):
                """All quarter-chunks of tile t into G[:, :CT[t]*ROW]."""
                for (q, c0, cl) in chunks[t]:
                    it = ipool.tile([P, MAXC * 8], dt.int16, tag="it")
                    nc.sync.dma_start(
                        out=it[:, :cl * 8],
                        in_=idx_in[:, wtoff[t] + c0 * 8:
                                   wtoff[t] + (c0 + cl) * 8])
                    nc.gpsimd.dma_gather(
                        out_ap=G[:, c0 * ROW:(c0 + cl) * ROW]
                            .rearrange("p (c w) -> p c w", w=ROW),
                        in_ap=full[q * qsz:(q + 1) * qsz, :],
                        idxs_ap=it[:, :cl * 8],
                        num_idxs=cl * P,
                        num_idxs_reg=cl * P,
                        elem_size=ROW,
                        single_packet=False,
                        queue_num=qrr[0] % NQUEUES,
                    )
                    qrr[0] += 1

            for _rep in range(repeat):
                # Shared scratch allows a single writer inst; allocate the
                # tables per repetition (identical program for repeat=1).
                t1_shard = dpool.tile([SH, ROW], dt.float16, name=f"t1s{_rep}")
                t1_full = dpool.tile([NROWS, ROW], dt.float16,
                                     addr_space="Shared", name=f"t1f{_rep}")
                t2_shard = dpool.tile([SH, ROW], dt.float16, name=f"t2s{_rep}")
                t2_full = dpool.tile([NROWS, ROW], dt.float16,
                                     addr_space="Shared", name=f"t2f{_rep}")
                # ---- phase A: h1 = x @ W1aug per 128-node tile ----
                for t in range(TILES):
                    ps = ppool.tile([P, 80], dt.float32, tag="psA")
                    lhs = wpool.tile([P, W1C * P], dt.float16, tag="xT")
                    nc.sync.dma_start(
                        out=lhs[:].rearrange("p (c n) -> p c n", n=P),
                        in_=xT[:, t * P:(t + 1) * P]
                            .rearrange("(c p) n -> p c n", p=P))
                    for c in range(W1C):
                        nc.tensor.matmul(
                            out=ps[:], lhsT=lhs[:, c * P:(c + 1) * P],
                            rhs=w1_t[:, c * 80:(c + 1) * 80],
                            start=(c == 0), stop=(c == W1C - 1))
                    row = wpool.tile([P, T1W], dt.float16, tag="t1row")
                    nc.scalar.copy(row[:], ps[:, 0:T1W])
                    nc.sync.dma_start(
                        out=t1_shard[t * P:(t + 1) * P, 0:T1W], in_=row[:])
                    nc.vector.tensor_copy(ald1[:, t * HEADS:(t + 1) * HEADS],
                                          ps[:, T1W:80])
                # dummy pad rows: al parts -> BIG_NEG so filler slots get p=0
                if npad > 0:
                    dummy = wpool.tile([P, T1W], dt.float16, tag="dummy")
                    nc.vector.memset(dummy[:], BIG_NEG)
                    nc.sync.dma_start(
                        out=t1_shard[SH - npad:SH, 0:T1W], in_=dummy[:npad, :])

                nc.gpsimd.collective_compute(
                    "AllGather", OP.bypass,
                    replica_groups=[list(range(CORES))],
                    ins=[t1_shard[:].opt()], outs=[t1_full[:].opt()])

                # ---- phase C1: layer-1 aggregation + layer-2 projection ----
                for t in range(TILES):
                    SL = int(CT[t])
                    G = gpool.tile([P, SL * ROW], dt.float16, tag="G1")
                    gather_tile(G, t, t1_full)
                    # attention logits: al_s[src] + al_d[dst]  [P, SL, HEADS]
                    plog = wpool.tile([P, SL * HEADS], dt.float32, tag="plog")
                    G_al = G[:].rearrange("p (d w) -> p d w", w=ROW)[:, :, 0:HEADS]
                    ald_b = ald1[:, t * HEADS:(t + 1) * HEADS].unsqueeze(1) \
                        .broadcast_to([P, SL, HEADS])
                    nc.vector.tensor_tensor(
                        out=plog[:].rearrange("p (d w) -> p d w", w=HEADS),
                        in0=G_al, in1=ald_b, op=OP.add)
                    nc.vector.scalar_tensor_tensor(
                        out=plog[:], in0=plog[:], scalar=NEG_SLOPE, in1=plog[:],
                        op0=OP.mult, op1=OP.max)
                    nc.scalar.activation(plog[:], plog[:], AF.Exp)
                    den = wpool.tile([P, HEADS], dt.float32, tag="den")
                    nc.vector.tensor_reduce(
                        out=den[:],
                        in_=plog[:].rearrange("p (d w) -> p w d", w=HEADS),
                        axis=AX.X, op=OP.add)
                    rec = wpool.tile([P, HEADS], dt.float32, tag="rec")
                    nc.vector.reciprocal(rec[:], den[:])
                    # alpha = p * (1/den): pre-normalized, fp16
                    alpha = wpool.tile([P, SL * HEADS], dt.float16, tag="alpha")
                    rec_b = rec[:].unsqueeze(1).broadcast_to([P, SL, HEADS])
                    nc.vector.tensor_tensor(
                        out=alpha[:].rearrange("p (d w) -> p d w", w=HEADS),
                        in0=plog[:].rearrange("p (d w) -> p d w", w=HEADS),
                        in1=rec_b, op=OP.mult)
                    # weighted features: Gp[p, d, h, f] = G_h * alpha
                    Gp = wpool.tile([P, SL * F_HID], dt.float16, tag="Gp")
                    G_h = G[:].rearrange("p (d w) -> p d w", w=ROW) \
                        [:, :, HEADS:T1W].rearrange("p d (h f) -> p d h f", f=HID)
                    a_b = alpha[:].rearrange("p (d h) -> p d h", h=HEADS) \
                        .unsqueeze(3).broadcast_to([P, SL, HEADS, HID])
                    nc.vector.tensor_tensor(
                        out=Gp[:].rearrange("p (d h f) -> p d h f",
                                            h=HEADS, f=HID),
                        in0=G_h, in1=a_b, op=OP.mult)
                    h2 = wpool.tile([P, F_HID], dt.float32, tag="h2")
                    nc.vector.tensor_reduce(
                        out=h2[:],
                        in_=Gp[:].rearrange("p (d w) -> p w d", w=F_HID),
                        axis=AX.X, op=OP.add)
                    # + bias, then elu
                    nc.vector.tensor_tensor(out=h2[:], in0=h2[:], in1=b1_t[:],
                                            op=OP.add)
                    mn = wpool.tile([P, F_HID], dt.float32, tag="mn")
                    nc.vector.tensor_scalar_min(mn[:], h2[:], 0.0)
                    nc.scalar.activation(mn[:], mn[:], AF.Exp)
                    nc.vector.scalar_tensor_tensor(
                        out=h2[:], in0=h2[:], scalar=0.0, in1=mn[:],
                        op0=OP.max, op1=OP.add)
                    nc.vector.tensor_scalar_add(h2[:], h2[:], -1.0)
                    # layer-2 projection: g = h2 @ W2aug (transpose h2 via PE)
                    pst = ppool.tile([F_HID, P], dt.float32, tag="psT")
                    nc.tensor.transpose(out=pst[:], in_=h2[:], identity=ident[:])
                    h2T = wpool.tile([F_HID, P], dt.float16, tag="h2T")
                    nc.scalar.copy(h2T[:], pst[:])
                    ps2 = ppool.tile([P, 42], dt.float32, tag="ps2")
                    nc.tensor.matmul(out=ps2[:], lhsT=h2T[:], rhs=w2_t[:],
                                     start=True, stop=True)
                    row2 = wpool.tile([P, T2W], dt.float16, tag="t2row")
                    nc.scalar.copy(row2[:], ps2[:, 0:T2W])
                    nc.sync.dma_start(
                        out=t2_shard[t * P:(t + 1) * P, 0:T2W], in_=row2[:])
                    nc.vector.tensor_copy(ald2[:, t:t + 1], ps2[:, T2W:42])
                if npad > 0:
                    dummy2 = wpool.tile([P, T2W], dt.float16, tag="dummy2")
                    nc.vector.memset(dummy2[:], BIG_NEG)
                    nc.sync.dma_start(
                        out=t2_shard[SH - npad:SH, 0:T2W], in_=dummy2[:npad, :])

                nc.gpsimd.collective_compute(
                    "AllGather", OP.bypass,
                    replica_groups=[list(range(CORES))],
                    ins=[t2_shard[:].opt()], outs=[t2_full[:].opt()])

                # ---- phase C2: layer-2 aggregation + log_softmax ----
                for t in range(TILES):
                    SL = int(CT[t])
                    G2 = gpool.tile([P, SL * ROW], dt.float16, tag="G2")
                    gather_tile(G2, t, t2_full)
                    p2 = wpool.tile([P, SL], dt.float32, tag="p2")
                    nc.vector.tensor_scalar(
                        out=p2[:],
                        in0=G2[:].rearrange("p (d w) -> p d w", w=ROW)
                            [:, :, 0:1].squeeze(2),
                        scalar1=ald2[:, t:t + 1], scalar2=None, op0=OP.add)
                    nc.vector.scalar_tensor_tensor(
                        out=p2[:], in0=p2[:], scalar=NEG_SLOPE, in1=p2[:],
                        op0=OP.mult, op1=OP.max)
                    den2 = wpool.tile([P, 1], dt.float32, tag="den2")
                    nc.scalar.activation(p2[:], p2[:], AF.Exp, accum_out=den2[:])
                    rec2 = wpool.tile([P, 1], dt.float32, tag="rec2")
                    nc.vector.reciprocal(rec2[:], den2[:])
                    alpha2 = wpool.tile([P, SL], dt.float16, tag="alpha2")
                    nc.vector.tensor_scalar(
                        out=alpha2[:], in0=p2[:], scalar1=rec2[:, 0:1],
                        scalar2=None, op0=OP.mult)
                    G2p = wpool.tile([P, SL * NUM_CLASSES], dt.float16, tag="G2p")
                    G2_h = G2[:].rearrange("p (d w) -> p d w", w=ROW)[:, :, 1:T2W]
                    a2_b = alpha2[:].unsqueeze(2).broadcast_to(
                        [P, SL, NUM_CLASSES])
                    nc.vector.tensor_tensor(
                        out=G2p[:].rearrange("p (d w) -> p d w", w=NUM_CLASSES),
                        in0=G2_h, in1=a2_b, op=OP.mult)
                    o2 = wpool.tile([P, NUM_CLASSES], dt.float32, tag="o2")
                    nc.vector.tensor_reduce(
                        out=o2[:],
                        in_=G2p[:].rearrange("p (d w) -> p w d", w=NUM_CLASSES),
                        axis=AX.X, op=OP.add)
                    nc.vector.tensor_tensor(out=o2[:], in0=o2[:], in1=b2_t[:],
                                            op=OP.add)
                    # log_softmax over classes
                    mx = wpool.tile([P, 1], dt.float32, tag="mx")
                    nc.vector.tensor_reduce(out=mx[:], in_=o2[:], axis=AX.X,
                                            op=OP.max)
                    nc.vector.tensor_scalar(out=o2[:], in0=o2[:],
                                            scalar1=mx[:, 0:1],
                                            scalar2=None, op0=OP.subtract)
                    ex = wpool.tile([P, NUM_CLASSES], dt.float32, tag="ex")
                    sm = wpool.tile([P, 1], dt.float32, tag="sm")
                    nc.scalar.activation(ex[:], o2[:], AF.Exp, accum_out=sm[:])
                    lg = wpool.tile([P, 1], dt.float32, tag="lg")
                    nc.scalar.activation(lg[:], sm[:], AF.Ln)
                    nc.vector.tensor_scalar(out=o2[:], in0=o2[:],
                                            scalar1=lg[:, 0:1],
                                            scalar2=None, op0=OP.subtract)
                    nc.sync.dma_start(out=out[t * P:(t + 1) * P, :], in_=o2[:])
    nc.compile()
    return nc


def _make_runner(nc, n_cores=CORES):
    """Hold a jitted PJRT executable for repeated invocation."""
    import jax
    from jax.sharding import Mesh, PartitionSpec
    from jax.experimental.shard_map import shard_map
    from concourse import mybir
    from concourse.bass2jax import (_bass_exec_p, install_neuronx_cc_hook,
                                    partition_id_tensor)
    install_neuronx_cc_hook()
    partition_name = nc.partition_id_tensor.name if nc.partition_id_tensor else None
    in_names, out_names, out_avals, zero_outs = [], [], [], []
    for alloc in nc.m.functions[0].allocations:
        if not isinstance(alloc, mybir.MemoryLocationSet):
            continue
        name = alloc.memorylocations[0].name
        if alloc.kind == "ExternalInput":
            if name != partition_name:
                in_names.append(name)
        elif alloc.kind == "ExternalOutput":
            shape = tuple(alloc.tensor_shape)
            dtype = mybir.dt.np(alloc.dtype)
            out_names.append(name)
            out_avals.append(jax.core.ShapedArray(shape, dtype))
            zero_outs.append(np.zeros(shape, dtype))
    n_params = len(in_names)
    all_in = list(in_names) + list(out_names) + ([partition_name] if partition_name else [])

    def _body(*args):
        operands = list(args)
        if partition_name is not None:
            operands.append(partition_id_tensor())
        outs = _bass_exec_p.bind(
            *operands, out_avals=tuple(out_avals), in_names=tuple(all_in),
            out_names=tuple(out_names), lowering_input_output_aliases=(),
            sim_require_finite=True, sim_require_nnan=True, nc=nc)
        return tuple(outs)

    devices = jax.devices()[:n_cores]
    mesh = Mesh(np.asarray(devices), ("core",))
    nio = n_params + len(out_names)
    sharded = jax.jit(
        shard_map(_body, mesh=mesh, in_specs=(PartitionSpec("core"),) * nio,
                  out_specs=(PartitionSpec("core"),) * len(out_names),
                  check_rep=False),
        keep_unused=True)

    def run(in_maps, time_reps=0):
        import time as _t
        concat_in = [np.concatenate([np.asarray(in_maps[c][nm])
                                     for c in range(n_cores)], axis=0)
                     for nm in in_names]
        concat_zero = [np.zeros((n_cores * z.shape[0], *z.shape[1:]), z.dtype)
                       for z in zero_outs]
        dev_in = [jax.device_put(a) for a in concat_in]
        dev_zero = [jax.device_put(a) for a in concat_zero]
        outs = sharded(*dev_in, *dev_zero)
        jax.block_until_ready(outs)
        tmin = None
        if time_reps:
            ts = []
            for _ in range(time_reps):
                t0 = _t.perf_counter()
                outs = sharded(*dev_in, *dev_zero)
                jax.block_until_ready(outs)
                ts.append(_t.perf_counter() - t0)
            tmin = min(ts)
        results = [{nm: np.asarray(outs[i]).reshape(n_cores, *out_avals[i].shape)[c]
                    for i, nm in enumerate(out_names)} for c in range(n_cores)]
        return results, tmin

    return run


def _prepare_inputs(x, W1, a_src1, a_dst1, b1, W2, a_src2, a_dst2, b2, meta):
    SH = meta["shard_pad"]
    As = np.zeros((F_HID, HEADS), dtype=np.float32)
    Ad = np.zeros((F_HID, HEADS), dtype=np.float32)
    for h in range(HEADS):
        As[h * HID:(h + 1) * HID, h] = a_src1[h]
        Ad[h * HID:(h + 1) * HID, h] = a_dst1[h]
    w1aug = np.concatenate([W1 @ As, W1, W1 @ Ad], axis=1).astype(np.float16)
    w2aug = np.concatenate([W2 @ a_src2.T, W2, W2 @ a_dst2.T],
                           axis=1).astype(np.float16)
    b1rep = np.broadcast_to(b1[None, :], (P, F_HID)).copy()
    b2rep = np.broadcast_to(b2[None, :], (P, NUM_CLASSES)).copy()

    in_maps = []
    for r in range(CORES):
        perm = meta["perms"][r]
        xs = np.zeros((SH, F_IN), dtype=np.float16)
        xs[:len(perm)] = x[perm]
        in_maps.append({
            "xT": np.ascontiguousarray(xs.T),
            "w1aug": w1aug, "w2aug": w2aug,
            "b1rep": b1rep, "b2rep": b2rep,
            "idx": meta["idx"][r],
        })
    return in_maps


def kernel(x, edge_index, W1, a_src1, a_dst1, b1, W2, a_src2, a_dst2, b2,
           _time_reps=0, _repeat=1):
    x = np.asarray(x, dtype=np.float32)
    edge_index = np.asarray(edge_index)
    W1 = np.asarray(W1, dtype=np.float32)
    W2 = np.asarray(W2, dtype=np.float32)
    a_src1 = np.asarray(a_src1, dtype=np.float32)
    a_dst1 = np.asarray(a_dst1, dtype=np.float32)
    a_src2 = np.asarray(a_src2, dtype=np.float32)
    a_dst2 = np.asarray(a_dst2, dtype=np.float32)
    b1 = np.asarray(b1, dtype=np.float32)
    b2 = np.asarray(b2, dtype=np.float32)

    meta = _preprocess_graph(edge_index)
    key = (tuple(meta["CT"].tolist()), meta["shard_pad"], _repeat)
    if key not in _PROGRAM_CACHE:
        nc = _build_program(meta, repeat=_repeat)
        _PROGRAM_CACHE[key] = _make_runner(nc)
    run = _PROGRAM_CACHE[key]

    in_maps = _prepare_inputs(x, W1, a_src1, a_dst1, b1, W2, a_src2, a_dst2,
                              b2, meta)
    try:
        results, tmin = run(in_maps, time_reps=_time_reps)
    except Exception:
        # transient axon/NRT failures have been observed; one retry is cheap
        results, tmin = run(in_maps, time_reps=_time_reps)
    out = np.zeros((N, NUM_CLASSES), dtype=np.float32)
    for r in range(CORES):
        perm = meta["perms"][r]
        out[perm] = results[r]["out"][:len(perm)]
    if _time_reps:
        kernel._last_time_s = tmin
    return out


# revision 7
# speedup vs baseline: 1.1678x; 1.1678x over previous
"""2-layer GAT (GATConv x2, PyG-style with self-loops) on 8 Trainium2 NeuronCores.

Strategy (graph/data parallel, per sharding hint):
- Nodes are sharded across 8 cores (12500 each, padded to 12544). Each core
  computes the layer projections for its node shard, with the attention
  vectors folded into the projection weights on the host
  (W_aug = [W@Asrc | W | W@Adst]), and writes a per-node 256-byte table row
  [al_src(8) | feat | pad] to DRAM. Tables are AllGathered (Shared
  scratchpad) so every core holds all source-node rows (halo).
- Incident edges are partitioned by destination; self-loops are ordinary
  edges. Per 128-destination tile, edges sit in slot columns (dest on
  partitions). Slots are fetched with nc.gpsimd.dma_gather
  (InstDMAGatherAnt, single_packet=False), whose Q7 ucode generates
  descriptors ~15x faster than the generic indirect-DMA path (measured
  ~0.7 vs ~11 ns/row): row i of a call lands on partition i%128, column
  i//128 - exactly the slot grid. dma_gather indices are int16, so the
  100352-row table is split into four 25088-row quarters; each tile's slot
  columns are grouped by source-quarter (padded per quarter to the max
  per-partition count over cores, ~2.4x column inflation) and fetched by
  per-quarter calls of <=896 rows (SWDGE descriptor-ring limit ~1024).
  Filler slots point at a pad row whose al_src is -60000, so exp()
  contributes exactly 0. Attention softmax and the weighted aggregation run
  as wide strided DVE/ACT ops over the gathered block; max-subtraction is
  skipped (logits are O(1), exp is stable, exp(e-m)/sum == exp(e)/sum).
  Alpha is pre-normalized (p * 1/den) in fp16 for 2x DVE throughput.

Host side does only index/graph preprocessing (shard, degree-sort, slot
packing, quarter split) plus weight folding; all FLOPs on the N-dimension
run on device.
"""

import numpy as np

# Problem constants (hardcoded per spec)
N = 100000
E = 1600000
F_IN = 512
HID = 8
HEADS = 8
F_HID = HID * HEADS  # 64
NUM_CLASSES = 40
NEG_SLOPE = 0.2
CORES = 8
P = 128
ROW = 128           # table row stride in fp16 elements (256 B, dma_gather min)
NQ = 4              # int16 index range => 4 table quarters
BIG_NEG = -60000.0  # al_src of dummy/pad rows; exp(leaky(.)) == 0.0 exactly
MAXC = 7            # max slot columns per dma_gather call (<=896 descriptors)
NQUEUES = 4         # SWDGE queues; round-robin parallelizes Q7 desc-gen ~4.7x

_PROGRAM_CACHE = {}


def _split_shards(n, cores):
    base = n // cores
    rem = n % cores
    sizes = [base + (1 if r < rem else 0) for r in range(cores)]
    offs = np.concatenate([[0], np.cumsum(sizes)])
    return sizes, offs


def _preprocess_graph(edge_index, n=N, cores=CORES):
    """Shard dst nodes, degree-sort within shard, pack per-quarter edge slots.

    Returns per-core wrapped int16 index arrays plus the shared tile/chunk
    structure (identical across cores so the SPMD program is uniform).
    """
    loops = np.arange(n, dtype=np.int64)
    src = np.concatenate([edge_index[0].astype(np.int64), loops])
    dst = np.concatenate([edge_index[1].astype(np.int64), loops])

    sizes, offs = _split_shards(n, cores)
    shard_pad = int(np.ceil((max(sizes) + 1) / P) * P)  # 12544
    tiles = shard_pad // P
    qsz = shard_pad * cores // NQ  # 25088 rows per quarter (= 2 shards)

    deg = np.bincount(dst, minlength=n)
    node_core = np.searchsorted(offs[1:], np.arange(n), side="right")

    perms = []
    node_pos = np.zeros(n, dtype=np.int64)
    for r in range(cores):
        own = np.arange(offs[r], offs[r + 1])
        order = own[np.argsort(-deg[own], kind="stable")]
        perms.append(order)
        node_pos[order] = np.arange(len(order))

    table_row = node_core * shard_pad + node_pos

    # per-(tile, quarter) slot counts, uniform across cores
    Cq = np.zeros((tiles, NQ), dtype=np.int64)
    per_core = []
    for r in range(cores):
        m = (dst >= offs[r]) & (dst < offs[r + 1])
        e_src = src[m]
        e_dst = dst[m]
        lpos = node_pos[e_dst]
        rows = table_row[e_src]
        q = rows // qsz
        t = lpos // P
        p = lpos % P
        cnt = np.zeros((tiles, P, NQ), dtype=np.int64)
        np.add.at(cnt, (t, p, q), 1)
        Cq = np.maximum(Cq, cnt.max(axis=1))
        per_core.append((e_src, lpos, rows, q, t, p))
    Cq[:, 0] = np.maximum(Cq[:, 0], 1)  # room for the pad-dst anchor (row 0)

    # column layout per tile: [q0 block | q1 | q2 | q3]
    qoff = np.concatenate([np.zeros((tiles, 1), np.int64),
                           np.cumsum(Cq, axis=1)], axis=1)  # [tiles, 5]
    CT = qoff[:, NQ]          # total columns per tile
    toff = np.concatenate([[0], np.cumsum(CT)])  # tile column offsets
    S = int(toff[-1])

    DUMMY_LOCAL = sizes[0]  # pad row of the quarter's low shard (= 12500)

    idx_arrays = []
    for r in range(cores):
        e_src, lpos, rows, q, t, p = per_core[r]
        grid = np.full((P, S), DUMMY_LOCAL, dtype=np.int64)
        # slot within (tile, partition, quarter), via stable sort by (q,t,p)
        order = np.lexsort((p, t, q))
        qq, tt, pp, rr = q[order], t[order], p[order], rows[order]
        key = (qq * tiles + tt) * P + pp
        grp_start = np.searchsorted(key, key, side="left")
        slot = np.arange(len(key)) - grp_start
        col = toff[tt] + qoff[tt, qq] + slot
        grid[pp, col] = rr - qq * qsz
        # padded dst rows: anchor slot 0 (a q0 column) to row 0
        n_real = sizes[r]
        if shard_pad > n_real:
            pl = np.arange(n_real, shard_pad)
            grid[pl % P, toff[pl // P]] = 0
        idx_arrays.append(grid.astype(np.int16))

    # chunk list per tile: (q, col_start_in_tile, n_cols)
    chunks = []
    for t in range(tiles):
        ch = []
        for q in range(NQ):
            c0 = int(qoff[t, q])
            left = int(Cq[t, q])
            while left > 0:
                cl = min(MAXC, left)
                ch.append((q, c0, cl))
                c0 += cl
                left -= cl
        chunks.append(ch)

    # wrapped idx DRAM image [P, sum_over_chunks(cl*8)] int16 per core:
    # per chunk, flat list (c-major, p-minor) wrapped 16-way, replicated 8x.
    wtoff = []  # per tile, start column (in int16 units) of its idx block
    wpos = 0
    for t in range(tiles):
        wtoff.append(wpos)
        wpos += int(CT[t]) * 8
    WTOT = wpos
    idx_wrapped = []
    for r in range(cores):
        grid = idx_arrays[r]
        img = np.zeros((P, WTOT), dtype=np.int16)
        for t in range(tiles):
            pos = wtoff[t]
            for (q, c0, cl) in chunks[t]:
                cols = grid[:, toff[t] + c0: toff[t] + c0 + cl]  # [P, cl]
                flat = cols.T.reshape(-1)                        # c-major
                wr = flat.reshape(cl * 8, 16).T                  # [16, cl*8]
                img[:, pos:pos + cl * 8] = np.tile(wr, (8, 1))
                pos += cl * 8
        idx_wrapped.append(img)

    return {
        "idx": idx_wrapped,
        "perms": perms,
        "sizes": sizes,
        "offs": offs,
        "shard_pad": shard_pad,
        "tiles": tiles,
        "Cq": Cq,
        "CT": CT,
        "toff": toff,
        "chunks": chunks,
        "wtoff": wtoff,
        "WTOT": WTOT,
        "qsz": qsz,
        "pad_rows_start": [sizes[r] for r in range(cores)],
    }


def _build_program(meta, repeat=1):
    """Build the SPMD Bass program (identical across cores)."""
    from concourse import mybir, bacc
    import concourse.tile as tile
    from concourse.masks import make_identity

    dt = mybir.dt
    SH = meta["shard_pad"]
    TILES = meta["tiles"]
    NROWS = SH * CORES
    W1C = F_IN // P
    T1W = HEADS + F_HID  # 72 used fp16 of each 128-elem row
    T2W = 1 + NUM_CLASSES  # 41
    CT = meta["CT"]
    chunks = meta["chunks"]
    wtoff = meta["wtoff"]
    WTOT = meta["WTOT"]
    qsz = meta["qsz"]

    nc = bacc.Bacc("TRN2", target_bir_lowering=False, debug=False,
                   num_devices=CORES, num_swdge_queues=NQUEUES)
    xT = nc.dram_tensor("xT", [F_IN, SH], dt.float16, kind="ExternalInput")
    w1aug = nc.dram_tensor("w1aug", [F_IN, 80], dt.float16, kind="ExternalInput")
    w2aug = nc.dram_tensor("w2aug", [F_HID, 42], dt.float16, kind="ExternalInput")
    b1rep = nc.dram_tensor("b1rep", [P, F_HID], dt.float32, kind="ExternalInput")
    b2rep = nc.dram_tensor("b2rep", [P, NUM_CLASSES], dt.float32, kind="ExternalInput")
    idx_in = nc.dram_tensor("idx", [P, WTOT], dt.int16, kind="ExternalInput")
    out = nc.dram_tensor("out", [SH, NUM_CLASSES], dt.float32, kind="ExternalOutput")

    AF = mybir.ActivationFunctionType
    OP = mybir.AluOpType
    AX = mybir.AxisListType

    with tile.TileContext(nc) as tc:
        with (
            tc.tile_pool(name="const", bufs=1) as cpool,
            tc.tile_pool(name="resident", bufs=1) as rpool,
            tc.tile_pool(name="work", bufs=3) as wpool,
            tc.tile_pool(name="gbuf", bufs=3) as gpool,
            tc.tile_pool(name="ibuf", bufs=3) as ipool,
            tc.tile_pool(name="psum", bufs=2, space="PSUM") as ppool,
            tc.tile_pool(name="dram", bufs=1, space="DRAM") as dpool,
        ):
            # ---- constants / residents ----
            w1_t = cpool.tile([P, W1C * 80], dt.float16)
            for c in range(W1C):
                nc.sync.dma_start(out=w1_t[:, c * 80:(c + 1) * 80],
                                  in_=w1aug[c * P:(c + 1) * P, :])
            w2_t = cpool.tile([F_HID, 42], dt.float16)
            nc.sync.dma_start(out=w2_t[:], in_=w2aug[:, :])
            b1_t = cpool.tile([P, F_HID], dt.float32)
            nc.sync.dma_start(out=b1_t[:], in_=b1rep[:, :])
            b2_t = cpool.tile([P, NUM_CLASSES], dt.float32)
            nc.sync.dma_start(out=b2_t[:], in_=b2rep[:, :])
            ident = cpool.tile([P, P], dt.float32)
            make_identity(nc, ident[:])
            ald1 = rpool.tile([P, TILES * HEADS], dt.float32)
            ald2 = rpool.tile([P, TILES], dt.float32)

            npad = SH - meta["pad_rows_start"][0]

            # 256B-stride gather tables (local; cols T*W..127 stay garbage,
            # never read). AllGather moves only the compact rows; a few big
            # strided HWDGE DMAs expand them into the gather stride.
            t1_full = dpool.tile([NROWS, ROW], dt.float16)
            t2_full = dpool.tile([NROWS, ROW], dt.float16)

            qrr = [0]  # round-robin queue counter (shared across layers)

            def gather_tile(G, t, full):
                """All quarter-chunks of tile t into G[:, :CT[t]*ROW]."""
                for (q, c0, cl) in chunks[t]:
                    it = ipool.tile([P, MAXC * 8], dt.int16, tag="it")
                    nc.sync.dma_start(
                        out=it[:, :cl * 8],
                        in_=idx_in[:, wtoff[t] + c0 * 8:
                                   wtoff[t] + (c0 + cl) * 8])
                    nc.gpsimd.dma_gather(
                        out_ap=G[:, c0 * ROW:(c0 + cl) * ROW]
                            .rearrange("p (c w) -> p c w", w=ROW),
                        in_ap=full[q * qsz:(q + 1) * qsz, :],
                        idxs_ap=it[:, :cl * 8],
                        num_idxs=cl * P,
                        num_idxs_reg=cl * P,
                        elem_size=ROW,
                        single_packet=False,
                        queue_num=qrr[0] % NQUEUES,
                    )
                    qrr[0] += 1

            for _rep in range(repeat):
                # Shared scratch allows a single writer inst; allocate the
                # AG outputs per repetition (identical program for repeat=1).
                t1_shard = dpool.tile([SH, T1W], dt.float16, name=f"t1s{_rep}")
                t1c_full = dpool.tile([NROWS, T1W], dt.float16,
                                      addr_space="Shared", name=f"t1f{_rep}")
                t2_shard = dpool.tile([SH, T2W], dt.float16, name=f"t2s{_rep}")
                t2c_full = dpool.tile([NROWS, T2W], dt.float16,
                                      addr_space="Shared", name=f"t2f{_rep}")
                # ---- phase A: h1 = x @ W1aug per 128-node tile ----
                for t in range(TILES):
                    ps = ppool.tile([P, 80], dt.float32, tag="psA")
                    lhs = wpool.tile([P, W1C * P], dt.float16, tag="xT")
                    nc.sync.dma_start(
                        out=lhs[:].rearrange("p (c n) -> p c n", n=P),
                        in_=xT[:, t * P:(t + 1) * P]
                            .rearrange("(c p) n -> p c n", p=P))
                    for c in range(W1C):
                        nc.tensor.matmul(
                            out=ps[:], lhsT=lhs[:, c * P:(c + 1) * P],
                            rhs=w1_t[:, c * 80:(c + 1) * 80],
                            start=(c == 0), stop=(c == W1C - 1))
                    row = wpool.tile([P, T1W], dt.float16, tag="t1row")
                    nc.scalar.copy(row[:], ps[:, 0:T1W])
                    nc.sync.dma_start(
                        out=t1_shard[t * P:(t + 1) * P, 0:T1W], in_=row[:])
                    nc.vector.tensor_copy(ald1[:, t * HEADS:(t + 1) * HEADS],
                                          ps[:, T1W:80])
                # dummy pad rows: al parts -> BIG_NEG so filler slots get p=0
                if npad > 0:
                    dummy = wpool.tile([P, T1W], dt.float16, tag="dummy")
                    nc.vector.memset(dummy[:], BIG_NEG)
                    nc.sync.dma_start(
                        out=t1_shard[SH - npad:SH, 0:T1W], in_=dummy[:npad, :])

                nc.gpsimd.collective_compute(
                    "AllGather", OP.bypass,
                    replica_groups=[list(range(CORES))],
                    ins=[t1_shard[:].opt()], outs=[t1c_full[:].opt()])
                for k in range(CORES):
                    nc.sync.dma_start(
                        out=t1_full[k * SH:(k + 1) * SH, 0:T1W],
                        in_=t1c_full[k * SH:(k + 1) * SH, :])

                # ---- phase C1: layer-1 aggregation + layer-2 projection ----
                for t in range(TILES):
                    SL = int(CT[t])
                    G = gpool.tile([P, SL * ROW], dt.float16, tag="G1")
                    gather_tile(G, t, t1_full)
                    # attention logits: al_s[src] + al_d[dst]  [P, SL, HEADS]
                    plog = wpool.tile([P, SL * HEADS], dt.float32, tag="plog")
                    G_al = G[:].rearrange("p (d w) -> p d w", w=ROW)[:, :, 0:HEADS]
                    ald_b = ald1[:, t * HEADS:(t + 1) * HEADS].unsqueeze(1) \
                        .broadcast_to([P, SL, HEADS])
                    nc.vector.tensor_tensor(
                        out=plog[:].rearrange("p (d w) -> p d w", w=HEADS),
                        in0=G_al, in1=ald_b, op=OP.add)
                    nc.vector.scalar_tensor_tensor(
                        out=plog[:], in0=plog[:], scalar=NEG_SLOPE, in1=plog[:],
                        op0=OP.mult, op1=OP.max)
                    nc.scalar.activation(plog[:], plog[:], AF.Exp)
                    den = wpool.tile([P, HEADS], dt.float32, tag="den")
                    nc.vector.tensor_reduce(
                        out=den[:],
                        in_=plog[:].rearrange("p (d w) -> p w d", w=HEADS),
                        axis=AX.X, op=OP.add)
                    rec = wpool.tile([P, HEADS], dt.float32, tag="rec")
                    nc.vector.reciprocal(rec[:], den[:])
                    # alpha = p * (1/den): pre-normalized, fp16
                    alpha = wpool.tile([P, SL * HEADS], dt.float16, tag="alpha")
                    rec_b = rec[:].unsqueeze(1).broadcast_to([P, SL, HEADS])
                    nc.vector.tensor_tensor(
                        out=alpha[:].rearrange("p (d w) -> p d w", w=HEADS),
                        in0=plog[:].rearrange("p (d w) -> p d w", w=HEADS),
                        in1=rec_b, op=OP.mult)
                    # weighted features: Gp[p, d, h, f] = G_h * alpha
                    Gp = wpool.tile([P, SL * F_HID], dt.float16, tag="Gp")
                    G_h = G[:].rearrange("p (d w) -> p d w", w=ROW) \
                        [:, :, HEADS:T1W].rearrange("p d (h f) -> p d h f", f=HID)
                    a_b = alpha[:].rearrange("p (d h) -> p d h", h=HEADS) \
                        .unsqueeze(3).broadcast_to([P, SL, HEADS, HID])
                    nc.vector.tensor_tensor(
                        out=Gp[:].rearrange("p (d h f) -> p d h f",
                                            h=HEADS, f=HID),
                        in0=G_h, in1=a_b, op=OP.mult)
                    h2 = wpool.tile([P, F_HID], dt.float32, tag="h2")
                    nc.vector.tensor_reduce(
                        out=h2[:],
                        in_=Gp[:].rearrange("p (d w) -> p w d", w=F_HID),
                        axis=AX.X, op=OP.add)
                    # + bias, then elu
                    nc.vector.tensor_tensor(out=h2[:], in0=h2[:], in1=b1_t[:],
                                            op=OP.add)
                    mn = wpool.tile([P, F_HID], dt.float32, tag="mn")
                    nc.vector.tensor_scalar_min(mn[:], h2[:], 0.0)
                    nc.scalar.activation(mn[:], mn[:], AF.Exp)
                    nc.vector.scalar_tensor_tensor(
                        out=h2[:], in0=h2[:], scalar=0.0, in1=mn[:],
                        op0=OP.max, op1=OP.add)
                    nc.vector.tensor_scalar_add(h2[:], h2[:], -1.0)
                    # layer-2 projection: g = h2 @ W2aug (transpose h2 via PE)
                    pst = ppool.tile([F_HID, P], dt.float32, tag="psT")
                    nc.tensor.transpose(out=pst[:], in_=h2[:], identity=ident[:])
                    h2T = wpool.tile([F_HID, P], dt.float16, tag="h2T")
                    nc.scalar.copy(h2T[:], pst[:])
                    ps2 = ppool.tile([P, 42], dt.float32, tag="ps2")
                    nc.tensor.matmul(out=ps2[:], lhsT=h2T[:], rhs=w2_t[:],
                                     start=True, stop=True)
                    row2 = wpool.tile([P, T2W], dt.float16, tag="t2row")
                    nc.scalar.copy(row2[:], ps2[:, 0:T2W])
                    nc.sync.dma_start(
                        out=t2_shard[t * P:(t + 1) * P, 0:T2W], in_=row2[:])
                    nc.vector.tensor_copy(ald2[:, t:t + 1], ps2[:, T2W:42])
                if npad > 0:
                    dummy2 = wpool.tile([P, T2W], dt.float16, tag="dummy2")
                    nc.vector.memset(dummy2[:], BIG_NEG)
                    nc.sync.dma_start(
                        out=t2_shard[SH - npad:SH, 0:T2W], in_=dummy2[:npad, :])

                nc.gpsimd.collective_compute(
                    "AllGather", OP.bypass,
                    replica_groups=[list(range(CORES))],
                    ins=[t2_shard[:].opt()], outs=[t2c_full[:].opt()])
                for k in range(CORES):
                    nc.sync.dma_start(
                        out=t2_full[k * SH:(k + 1) * SH, 0:T2W],
                        in_=t2c_full[k * SH:(k + 1) * SH, :])

                # ---- phase C2: layer-2 aggregation + log_softmax ----
                for t in range(TILES):
                    SL = int(CT[t])
                    G2 = gpool.tile([P, SL * ROW], dt.float16, tag="G2")
                    gather_tile(G2, t, t2_full)
                    p2 = wpool.tile([P, SL], dt.float32, tag="p2")
                    nc.vector.tensor_scalar(
                        out=p2[:],
                        in0=G2[:].rearrange("p (d w) -> p d w", w=ROW)
                            [:, :, 0:1].squeeze(2),
                        scalar1=ald2[:, t:t + 1], scalar2=None, op0=OP.add)
                    nc.vector.scalar_tensor_tensor(
                        out=p2[:], in0=p2[:], scalar=NEG_SLOPE, in1=p2[:],
                        op0=OP.mult, op1=OP.max)
                    den2 = wpool.tile([P, 1], dt.float32, tag="den2")
                    nc.scalar.activation(p2[:], p2[:], AF.Exp, accum_out=den2[:])
                    rec2 = wpool.tile([P, 1], dt.float32, tag="rec2")
                    nc.vector.reciprocal(rec2[:], den2[:])
                    alpha2 = wpool.tile([P, SL], dt.float16, tag="alpha2")
                    nc.vector.tensor_scalar(
                        out=alpha2[:], in0=p2[:], scalar1=rec2[:, 0:1],
                        scalar2=None, op0=OP.mult)
                    G2p = wpool.tile([P, SL * NUM_CLASSES], dt.float16, tag="G2p")
                    G2_h = G2[:].rearrange("p (d w) -> p d w", w=ROW)[:, :, 1:T2W]
                    a2_b = alpha2[:].unsqueeze(2).broadcast_to(
                        [P, SL, NUM_CLASSES])
                    nc.vector.tensor_tensor(
                        out=G2p[:].rearrange("p (d w) -> p d w", w=NUM_CLASSES),
                        in0=G2_h, in1=a2_b, op=OP.mult)
                    o2 = wpool.tile([P, NUM_CLASSES], dt.float32, tag="o2")
                    nc.vector.tensor_reduce(
                        out=o2[:],
                        in_=G2p[:].rearrange("p (d w) -> p w d", w=NUM_CLASSES),
                        axis=AX.X, op=OP.add)
                    nc.vector.tensor_tensor(out=o2[:], in0=o2[:], in1=b2_t[:],
                                            op=OP.add)
                    # log_softmax over classes
                    mx = wpool.tile([P, 1], dt.float32, tag="mx")
                    nc.vector.tensor_reduce(out=mx[:], in_=o2[:], axis=AX.X,
                                            op=OP.max)
                    nc.vector.tensor_scalar(out=o2[:], in0=o2[:],
                                            scalar1=mx[:, 0:1],
                                            scalar2=None, op0=OP.subtract)
                    ex = wpool.tile([P, NUM_CLASSES], dt.float32, tag="ex")
                    sm = wpool.tile([P, 1], dt.float32, tag="sm")
                    nc.scalar.activation(ex[:], o2[:], AF.Exp, accum_out=sm[:])
                    lg = wpool.tile([P, 1], dt.float32, tag="lg")
                    nc.scalar.activation(lg[:], sm[:], AF.Ln)
                    nc.vector.tensor_scalar(out=o2[:], in0=o2[:],
                                            scalar1=lg[:, 0:1],
                                            scalar2=None, op0=OP.subtract)
                    nc.sync.dma_start(out=out[t * P:(t + 1) * P, :], in_=o2[:])
    nc.compile()
    return nc


def _make_runner(nc, n_cores=CORES):
    """Hold a jitted PJRT executable for repeated invocation."""
    import jax
    from jax.sharding import Mesh, PartitionSpec
    from jax.experimental.shard_map import shard_map
    from concourse import mybir
    from concourse.bass2jax import (_bass_exec_p, install_neuronx_cc_hook,
                                    partition_id_tensor)
    install_neuronx_cc_hook()
    partition_name = nc.partition_id_tensor.name if nc.partition_id_tensor else None
    in_names, out_names, out_avals, zero_outs = [], [], [], []
    for alloc in nc.m.functions[0].allocations:
        if not isinstance(alloc, mybir.MemoryLocationSet):
            continue
        name = alloc.memorylocations[0].name
        if alloc.kind == "ExternalInput":
            if name != partition_name:
                in_names.append(name)
        elif alloc.kind == "ExternalOutput":
            shape = tuple(alloc.tensor_shape)
            dtype = mybir.dt.np(alloc.dtype)
            out_names.append(name)
            out_avals.append(jax.core.ShapedArray(shape, dtype))
            zero_outs.append(np.zeros(shape, dtype))
    n_params = len(in_names)
    all_in = list(in_names) + list(out_names) + ([partition_name] if partition_name else [])

    def _body(*args):
        operands = list(args)
        if partition_name is not None:
            operands.append(partition_id_tensor())
        outs = _bass_exec_p.bind(
            *operands, out_avals=tuple(out_avals), in_names=tuple(all_in),
            out_names=tuple(out_names), lowering_input_output_aliases=(),
            sim_require_finite=True, sim_require_nnan=True, nc=nc)
        return tuple(outs)

    devices = jax.devices()[:n_cores]
    mesh = Mesh(np.asarray(devices), ("core",))
    nio = n_params + len(out_names)
    sharded = jax.jit(
        shard_map(_body, mesh=mesh, in_specs=(PartitionSpec("core"),) * nio,
                  out_specs=(PartitionSpec("core"),) * len(out_names),
                  check_rep=False),
        keep_unused=True)

    def run(in_maps, time_reps=0):
        import time as _t
        concat_in = [np.concatenate([np.asarray(in_maps[c][nm])
                                     for c in range(n_cores)], axis=0)
                     for nm in in_names]
        concat_zero = [np.zeros((n_cores * z.shape[0], *z.shape[1:]), z.dtype)
                       for z in zero_outs]
        dev_in = [jax.device_put(a) for a in concat_in]
        dev_zero = [jax.device_put(a) for a in concat_zero]
        outs = sharded(*dev_in, *dev_zero)
        jax.block_until_ready(outs)
        tmin = None
        if time_reps:
            ts = []
            for _ in range(time_reps):
                t0 = _t.perf_counter()
                outs = sharded(*dev_in, *dev_zero)
                jax.block_until_ready(outs)
                ts.append(_t.perf_counter() - t0)
            tmin = min(ts)
        results = [{nm: np.asarray(outs[i]).reshape(n_cores, *out_avals[i].shape)[c]
                    for i, nm in enumerate(out_names)} for c in range(n_cores)]
        return results, tmin

    return run


def _prepare_inputs(x, W1, a_src1, a_dst1, b1, W2, a_src2, a_dst2, b2, meta):
    SH = meta["shard_pad"]
    As = np.zeros((F_HID, HEADS), dtype=np.float32)
    Ad = np.zeros((F_HID, HEADS), dtype=np.float32)
    for h in range(HEADS):
        As[h * HID:(h + 1) * HID, h] = a_src1[h]
        Ad[h * HID:(h + 1) * HID, h] = a_dst1[h]
    w1aug = np.concatenate([W1 @ As, W1, W1 @ Ad], axis=1).astype(np.float16)
    w2aug = np.concatenate([W2 @ a_src2.T, W2, W2 @ a_dst2.T],
                           axis=1).astype(np.float16)
    b1rep = np.broadcast_to(b1[None, :], (P, F_HID)).copy()
    b2rep = np.broadcast_to(b2[None, :], (P, NUM_CLASSES)).copy()

    in_maps = []
    for r in range(CORES):
        perm = meta["perms"][r]
        xs = np.zeros((SH, F_IN), dtype=np.float16)
        xs[:len(perm)] = x[perm]
        in_maps.append({
            "xT": np.ascontiguousarray(xs.T),
            "w1aug": w1aug, "w2aug": w2aug,
            "b1rep": b1rep, "b2rep": b2rep,
            "idx": meta["idx"][r],
        })
    return in_maps


def kernel(x, edge_index, W1, a_src1, a_dst1, b1, W2, a_src2, a_dst2, b2,
           _time_reps=0, _repeat=1):
    x = np.asarray(x, dtype=np.float32)
    edge_index = np.asarray(edge_index)
    W1 = np.asarray(W1, dtype=np.float32)
    W2 = np.asarray(W2, dtype=np.float32)
    a_src1 = np.asarray(a_src1, dtype=np.float32)
    a_dst1 = np.asarray(a_dst1, dtype=np.float32)
    a_src2 = np.asarray(a_src2, dtype=np.float32)
    a_dst2 = np.asarray(a_dst2, dtype=np.float32)
    b1 = np.asarray(b1, dtype=np.float32)
    b2 = np.asarray(b2, dtype=np.float32)

    meta = _preprocess_graph(edge_index)
    key = (tuple(meta["CT"].tolist()), meta["shard_pad"], _repeat)
    if key not in _PROGRAM_CACHE:
        nc = _build_program(meta, repeat=_repeat)
        _PROGRAM_CACHE[key] = _make_runner(nc)
    run = _PROGRAM_CACHE[key]

    in_maps = _prepare_inputs(x, W1, a_src1, a_dst1, b1, W2, a_src2, a_dst2,
                              b2, meta)
    try:
        results, tmin = run(in_maps, time_reps=_time_reps)
    except Exception:
        # transient axon/NRT failures have been observed; one retry is cheap
        results, tmin = run(in_maps, time_reps=_time_reps)
    out = np.zeros((N, NUM_CLASSES), dtype=np.float32)
    for r in range(CORES):
        perm = meta["perms"][r]
        out[perm] = results[r]["out"][:len(perm)]
    if _time_reps:
        kernel._last_time_s = tmin
    return out


# revision 12
# speedup vs baseline: 2.0371x; 1.7443x over previous
"""2-layer GAT (GATConv x2, PyG-style with self-loops) on 8 Trainium2 NeuronCores.

Strategy (graph/data parallel, per sharding hint):
- Nodes are sharded across 8 cores (12500 each, padded to 12544). Each core
  computes the layer projections for its node shard, with the attention
  vectors folded into the projection weights on the host
  (W_aug = [W@Asrc | W | W@Adst]), and writes a per-node 256-byte table row
  [al_src(8) | feat | pad] to DRAM. Compact (unpadded) tables are
  AllGathered (Shared scratchpad) so every core holds all source-node rows
  (halo), then expanded locally into 256B-stride gather tables by a few big
  strided HWDGE DMAs (the AG link rate is ~17 GB/s, so gathering the padded
  rows directly would cost ~1.7 ms more).
- Incident edges are partitioned by destination; self-loops are ordinary
  edges. Per 128-destination tile, edges sit in slot columns (dest on
  partitions). Slots are fetched with nc.gpsimd.dma_gather
  (InstDMAGatherAnt, single_packet=False), whose Q7 ucode generates
  descriptors ~15x faster than the generic indirect-DMA path (measured
  ~0.7 vs ~11 ns/row): row i of a call lands on partition i%128, column
  i//128 - exactly the slot grid. dma_gather indices are int16, so the
  100352-row table is split into four 25088-row quarters; each tile's slot
  columns are grouped by source-quarter (padded per quarter to the max
  per-partition count over cores, ~2.4x column inflation) and fetched by
  per-quarter calls of <=896 rows (SWDGE descriptor-ring limit ~1024).
  Filler slots point at a pad row whose al_src is -60000, so exp()
  contributes exactly 0. Attention softmax and the weighted aggregation run
  as wide strided DVE/ACT ops over the gathered block; max-subtraction is
  skipped (logits are O(1), exp is stable, exp(e-m)/sum == exp(e)/sum).
  Alpha is pre-normalized (p * 1/den) in fp16 for 2x DVE throughput.

Host side does only index/graph preprocessing (shard, degree-sort, slot
packing, quarter split) plus weight folding; all FLOPs on the N-dimension
run on device.
"""

import numpy as np

# Problem constants (hardcoded per spec)
N = 100000
E = 1600000
F_IN = 512
HID = 8
HEADS = 8
F_HID = HID * HEADS  # 64
NUM_CLASSES = 40
NEG_SLOPE = 0.2
CORES = 8
P = 128
ROW = 128           # table row stride in fp16 elements (256 B, dma_gather min)
NQ = 4              # int16 index range => 4 table quarters
BIG_NEG = -60000.0  # al_src of dummy/pad rows; exp(leaky(.)) == 0.0 exactly
MAXC = 7            # max slot columns per dma_gather call (<=896 descriptors)
NQUEUES = 4         # SWDGE queues; round-robin parallelizes Q7 desc-gen ~4.7x

_PROGRAM_CACHE = {}


def _split_shards(n, cores):
    base = n // cores
    rem = n % cores
    sizes = [base + (1 if r < rem else 0) for r in range(cores)]
    offs = np.concatenate([[0], np.cumsum(sizes)])
    return sizes, offs


def _preprocess_graph(edge_index, n=N, cores=CORES):
    """Shard dst nodes, degree-sort within shard, pack per-quarter edge slots.

    Returns per-core wrapped int16 index arrays plus the shared tile/chunk
    structure (identical across cores so the SPMD program is uniform).
    """
    loops = np.arange(n, dtype=np.int64)
    src = np.concatenate([edge_index[0].astype(np.int64), loops])
    dst = np.concatenate([edge_index[1].astype(np.int64), loops])

    sizes, offs = _split_shards(n, cores)
    shard_pad = int(np.ceil((max(sizes) + 1) / P) * P)  # 12544
    tiles = shard_pad // P
    qsz = shard_pad * cores // NQ  # 25088 rows per quarter (= 2 shards)

    deg = np.bincount(dst, minlength=n)
    node_core = np.searchsorted(offs[1:], np.arange(n), side="right")

    perms = []
    node_pos = np.zeros(n, dtype=np.int64)
    for r in range(cores):
        own = np.arange(offs[r], offs[r + 1])
        order = own[np.argsort(-deg[own], kind="stable")]
        perms.append(order)
        node_pos[order] = np.arange(len(order))

    table_row = node_core * shard_pad + node_pos

    # per-(tile, quarter) slot counts, uniform across cores
    Cq = np.zeros((tiles, NQ), dtype=np.int64)
    per_core = []
    for r in range(cores):
        m = (dst >= offs[r]) & (dst < offs[r + 1])
        e_src = src[m]
        e_dst = dst[m]
        lpos = node_pos[e_dst]
        rows = table_row[e_src]
        q = rows // qsz
        t = lpos // P
        p = lpos % P
        cnt = np.zeros((tiles, P, NQ), dtype=np.int64)
        np.add.at(cnt, (t, p, q), 1)
        Cq = np.maximum(Cq, cnt.max(axis=1))
        per_core.append((e_src, lpos, rows, q, t, p))
    Cq[:, 0] = np.maximum(Cq[:, 0], 1)  # room for the pad-dst anchor (row 0)

    # column layout per tile: [q0 block | q1 | q2 | q3]
    qoff = np.concatenate([np.zeros((tiles, 1), np.int64),
                           np.cumsum(Cq, axis=1)], axis=1)  # [tiles, 5]
    CT = qoff[:, NQ]          # total columns per tile
    toff = np.concatenate([[0], np.cumsum(CT)])  # tile column offsets
    S = int(toff[-1])

    DUMMY_LOCAL = sizes[0]  # pad row of the quarter's low shard (= 12500)

    idx_arrays = []
    for r in range(cores):
        e_src, lpos, rows, q, t, p = per_core[r]
        grid = np.full((P, S), DUMMY_LOCAL, dtype=np.int64)
        # slot within (tile, partition, quarter), via stable sort by (q,t,p)
        order = np.lexsort((p, t, q))
        qq, tt, pp, rr = q[order], t[order], p[order], rows[order]
        key = (qq * tiles + tt) * P + pp
        grp_start = np.searchsorted(key, key, side="left")
        slot = np.arange(len(key)) - grp_start
        col = toff[tt] + qoff[tt, qq] + slot
        grid[pp, col] = rr - qq * qsz
        # padded dst rows: anchor slot 0 (a q0 column) to row 0
        n_real = sizes[r]
        if shard_pad > n_real:
            pl = np.arange(n_real, shard_pad)
            grid[pl % P, toff[pl // P]] = 0
        idx_arrays.append(grid.astype(np.int16))

    # chunk list per tile: (q, col_start_in_tile, n_cols)
    chunks = []
    for t in range(tiles):
        ch = []
        for q in range(NQ):
            c0 = int(qoff[t, q])
            left = int(Cq[t, q])
            while left > 0:
                cl = min(MAXC, left)
                ch.append((q, c0, cl))
                c0 += cl
                left -= cl
        chunks.append(ch)

    # wrapped idx DRAM image [P, sum_over_chunks(cl*8)] int16 per core:
    # per chunk, flat list (c-major, p-minor) wrapped 16-way, replicated 8x.
    wtoff = []  # per tile, start column (in int16 units) of its idx block
    wpos = 0
    for t in range(tiles):
        wtoff.append(wpos)
        wpos += int(CT[t]) * 8
    WTOT = wpos
    idx_wrapped = []
    for r in range(cores):
        grid = idx_arrays[r]
        img = np.zeros((P, WTOT), dtype=np.int16)
        for t in range(tiles):
            pos = wtoff[t]
            for (q, c0, cl) in chunks[t]:
                cols = grid[:, toff[t] + c0: toff[t] + c0 + cl]  # [P, cl]
                flat = cols.T.reshape(-1)                        # c-major
                wr = flat.reshape(cl * 8, 16).T                  # [16, cl*8]
                img[:, pos:pos + cl * 8] = np.tile(wr, (8, 1))
                pos += cl * 8
        idx_wrapped.append(img)

    return {
        "idx": idx_wrapped,
        "perms": perms,
        "sizes": sizes,
        "offs": offs,
        "shard_pad": shard_pad,
        "tiles": tiles,
        "Cq": Cq,
        "CT": CT,
        "toff": toff,
        "chunks": chunks,
        "wtoff": wtoff,
        "WTOT": WTOT,
        "qsz": qsz,
        "pad_rows_start": [sizes[r] for r in range(cores)],
    }


def _build_program(meta, repeat=1):
    """Build the SPMD Bass program (identical across cores)."""
    from concourse import mybir, bacc
    import concourse.tile as tile
    from concourse.masks import make_identity

    dt = mybir.dt
    SH = meta["shard_pad"]
    TILES = meta["tiles"]
    NROWS = SH * CORES
    W1C = F_IN // P
    T1W = HEADS + F_HID  # 72 used fp16 of each 128-elem row
    T2W = 1 + NUM_CLASSES  # 41
    CT = meta["CT"]
    chunks = meta["chunks"]
    wtoff = meta["wtoff"]
    WTOT = meta["WTOT"]
    qsz = meta["qsz"]

    nc = bacc.Bacc("TRN2", target_bir_lowering=False, debug=False,
                   num_devices=CORES, num_swdge_queues=NQUEUES)
    xT = nc.dram_tensor("xT", [F_IN, SH], dt.float16, kind="ExternalInput")
    w1aug = nc.dram_tensor("w1aug", [F_IN, 80], dt.float16, kind="ExternalInput")
    w2aug = nc.dram_tensor("w2aug", [F_HID, 42], dt.float16, kind="ExternalInput")
    b1rep = nc.dram_tensor("b1rep", [P, F_HID], dt.float32, kind="ExternalInput")
    b2rep = nc.dram_tensor("b2rep", [P, NUM_CLASSES], dt.float32, kind="ExternalInput")
    idx_in = nc.dram_tensor("idx", [P, WTOT], dt.int16, kind="ExternalInput")
    out = nc.dram_tensor("out", [SH, NUM_CLASSES], dt.float32, kind="ExternalOutput")

    AF = mybir.ActivationFunctionType
    OP = mybir.AluOpType
    AX = mybir.AxisListType

    with tile.TileContext(nc) as tc:
        with (
            tc.tile_pool(name="const", bufs=1) as cpool,
            tc.tile_pool(name="resident", bufs=1) as rpool,
            tc.tile_pool(name="work", bufs=3) as wpool,
            tc.tile_pool(name="gbuf", bufs=2) as gpool,
            tc.tile_pool(name="psum", bufs=2, space="PSUM") as ppool,
            tc.tile_pool(name="dram", bufs=1, space="DRAM") as dpool,
        ):
            # ---- constants / residents ----
            w1_t = cpool.tile([P, W1C * 80], dt.float16)
            for c in range(W1C):
                nc.sync.dma_start(out=w1_t[:, c * 80:(c + 1) * 80],
                                  in_=w1aug[c * P:(c + 1) * P, :])
            w2_t = cpool.tile([F_HID, 42], dt.float16)
            nc.sync.dma_start(out=w2_t[:], in_=w2aug[:, :])
            b1_t = cpool.tile([P, F_HID], dt.float32)
            nc.sync.dma_start(out=b1_t[:], in_=b1rep[:, :])
            b2_t = cpool.tile([P, NUM_CLASSES], dt.float32)
            nc.sync.dma_start(out=b2_t[:], in_=b2rep[:, :])
            ident = cpool.tile([P, P], dt.float32)
            make_identity(nc, ident[:])
            ald1 = rpool.tile([P, TILES * HEADS], dt.float32)
            ald2 = rpool.tile([P, TILES], dt.float32)
            # whole wrapped idx image resident (~66KB/partition): removes
            # ~1.5k per-chunk staging DMAs from the steady-state loop
            idx_t = rpool.tile([P, WTOT], dt.int16)
            nc.sync.dma_start(out=idx_t[:], in_=idx_in[:, :])

            npad = SH - meta["pad_rows_start"][0]

            # 256B-stride gather tables (local; cols T*W..127 stay garbage,
            # never read). AllGather moves only the compact rows; a few big
            # strided HWDGE DMAs expand them into the gather stride.
            t1_full = dpool.tile([NROWS, ROW], dt.float16)
            t2_full = dpool.tile([NROWS, ROW], dt.float16)

            qrr = [0]  # round-robin queue counter (shared across layers)

            def gather_tile(G, t, full):
                """All quarter-chunks of tile t into G[:, :CT[t]*ROW]."""
                for (q, c0, cl) in chunks[t]:
                    nc.gpsimd.dma_gather(
                        out_ap=G[:, c0 * ROW:(c0 + cl) * ROW]
                            .rearrange("p (c w) -> p c w", w=ROW),
                        in_ap=full[q * qsz:(q + 1) * qsz, :],
                        idxs_ap=idx_t[:, wtoff[t] + c0 * 8:
                                      wtoff[t] + (c0 + cl) * 8],
                        num_idxs=cl * P,
                        num_idxs_reg=cl * P,
                        elem_size=ROW,
                        single_packet=False,
                        queue_num=qrr[0] % NQUEUES,
                    )
                    qrr[0] += 1

            for _rep in range(repeat):
                # Shared scratch allows a single writer inst; allocate the
                # AG outputs per repetition (identical program for repeat=1).
                t1_shard = dpool.tile([SH, T1W], dt.float16, name=f"t1s{_rep}")
                t1c_full = dpool.tile([NROWS, T1W], dt.float16,
                                      addr_space="Shared", name=f"t1f{_rep}")
                t2_shard = dpool.tile([SH, T2W], dt.float16, name=f"t2s{_rep}")
                t2c_full = dpool.tile([NROWS, T2W], dt.float16,
                                      addr_space="Shared", name=f"t2f{_rep}")
                # ---- phase A: h1 = x @ W1aug per 128-node tile ----
                for t in range(TILES):
                    ps = ppool.tile([P, 80], dt.float32, tag="psA")
                    lhs = wpool.tile([P, W1C * P], dt.float16, tag="xT")
                    nc.sync.dma_start(
                        out=lhs[:].rearrange("p (c n) -> p c n", n=P),
                        in_=xT[:, t * P:(t + 1) * P]
                            .rearrange("(c p) n -> p c n", p=P))
                    for c in range(W1C):
                        nc.tensor.matmul(
                            out=ps[:], lhsT=lhs[:, c * P:(c + 1) * P],
                            rhs=w1_t[:, c * 80:(c + 1) * 80],
                            start=(c == 0), stop=(c == W1C - 1))
                    row = wpool.tile([P, T1W], dt.float16, tag="t1row")
                    nc.scalar.copy(row[:], ps[:, 0:T1W])
                    nc.sync.dma_start(
                        out=t1_shard[t * P:(t + 1) * P, 0:T1W], in_=row[:])
                    nc.vector.tensor_copy(ald1[:, t * HEADS:(t + 1) * HEADS],
                                          ps[:, T1W:80])
                # dummy pad rows: al parts -> BIG_NEG so filler slots get p=0
                if npad > 0:
                    dummy = wpool.tile([P, T1W], dt.float16, tag="dummy")
                    nc.vector.memset(dummy[:], BIG_NEG)
                    nc.sync.dma_start(
                        out=t1_shard[SH - npad:SH, 0:T1W], in_=dummy[:npad, :])

                nc.gpsimd.collective_compute(
                    "AllGather", OP.bypass,
                    replica_groups=[list(range(CORES))],
                    ins=[t1_shard[:].opt()], outs=[t1c_full[:].opt()])
                for k in range(CORES):
                    nc.sync.dma_start(
                        out=t1_full[k * SH:(k + 1) * SH, 0:T1W],
                        in_=t1c_full[k * SH:(k + 1) * SH, :])

                # ---- phase C1: layer-1 aggregation + layer-2 projection ----
                for t in range(TILES):
                    SL = int(CT[t])
                    G = gpool.tile([P, SL * ROW], dt.float16, tag="G1")
                    gather_tile(G, t, t1_full)
                    # attention logits: al_s[src] + al_d[dst]  [P, SL, HEADS]
                    plog = wpool.tile([P, SL * HEADS], dt.float32, tag="plog")
                    G_al = G[:].rearrange("p (d w) -> p d w", w=ROW)[:, :, 0:HEADS]
                    ald_b = ald1[:, t * HEADS:(t + 1) * HEADS].unsqueeze(1) \
                        .broadcast_to([P, SL, HEADS])
                    nc.vector.tensor_tensor(
                        out=plog[:].rearrange("p (d w) -> p d w", w=HEADS),
                        in0=G_al, in1=ald_b, op=OP.add)
                    nc.vector.scalar_tensor_tensor(
                        out=plog[:], in0=plog[:], scalar=NEG_SLOPE, in1=plog[:],
                        op0=OP.mult, op1=OP.max)
                    nc.scalar.activation(plog[:], plog[:], AF.Exp)
                    den = wpool.tile([P, HEADS], dt.float32, tag="den")
                    nc.vector.tensor_reduce(
                        out=den[:],
                        in_=plog[:].rearrange("p (d w) -> p w d", w=HEADS),
                        axis=AX.X, op=OP.add)
                    rec = wpool.tile([P, HEADS], dt.float32, tag="rec")
                    nc.vector.reciprocal(rec[:], den[:])
                    # alpha = p * (1/den): pre-normalized, fp16
                    alpha = wpool.tile([P, SL * HEADS], dt.float16, tag="alpha")
                    rec_b = rec[:].unsqueeze(1).broadcast_to([P, SL, HEADS])
                    nc.vector.tensor_tensor(
                        out=alpha[:].rearrange("p (d w) -> p d w", w=HEADS),
                        in0=plog[:].rearrange("p (d w) -> p d w", w=HEADS),
                        in1=rec_b, op=OP.mult)
                    # weighted features: Gp[p, d, h, f] = G_h * alpha
                    Gp = wpool.tile([P, SL * F_HID], dt.float16, tag="Gp")
                    G_h = G[:].rearrange("p (d w) -> p d w", w=ROW) \
                        [:, :, HEADS:T1W].rearrange("p d (h f) -> p d h f", f=HID)
                    a_b = alpha[:].rearrange("p (d h) -> p d h", h=HEADS) \
                        .unsqueeze(3).broadcast_to([P, SL, HEADS, HID])
                    nc.vector.tensor_tensor(
                        out=Gp[:].rearrange("p (d h f) -> p d h f",
                                            h=HEADS, f=HID),
                        in0=G_h, in1=a_b, op=OP.mult)
                    h2 = wpool.tile([P, F_HID], dt.float32, tag="h2")
                    nc.vector.tensor_reduce(
                        out=h2[:],
                        in_=Gp[:].rearrange("p (d w) -> p w d", w=F_HID),
                        axis=AX.X, op=OP.add)
                    # + bias, then elu
                    nc.vector.tensor_tensor(out=h2[:], in0=h2[:], in1=b1_t[:],
                                            op=OP.add)
                    mn = wpool.tile([P, F_HID], dt.float32, tag="mn")
                    nc.vector.tensor_scalar_min(mn[:], h2[:], 0.0)
                    nc.scalar.activation(mn[:], mn[:], AF.Exp)
                    nc.vector.scalar_tensor_tensor(
                        out=h2[:], in0=h2[:], scalar=0.0, in1=mn[:],
                        op0=OP.max, op1=OP.add)
                    nc.vector.tensor_scalar_add(h2[:], h2[:], -1.0)
                    # layer-2 projection: g = h2 @ W2aug (transpose h2 via PE)
                    pst = ppool.tile([F_HID, P], dt.float32, tag="psT")
                    nc.tensor.transpose(out=pst[:], in_=h2[:], identity=ident[:])
                    h2T = wpool.tile([F_HID, P], dt.float16, tag="h2T")
                    nc.scalar.copy(h2T[:], pst[:])
                    ps2 = ppool.tile([P, 42], dt.float32, tag="ps2")
                    nc.tensor.matmul(out=ps2[:], lhsT=h2T[:], rhs=w2_t[:],
                                     start=True, stop=True)
                    row2 = wpool.tile([P, T2W], dt.float16, tag="t2row")
                    nc.scalar.copy(row2[:], ps2[:, 0:T2W])
                    nc.sync.dma_start(
                        out=t2_shard[t * P:(t + 1) * P, 0:T2W], in_=row2[:])
                    nc.vector.tensor_copy(ald2[:, t:t + 1], ps2[:, T2W:42])
                if npad > 0:
                    dummy2 = wpool.tile([P, T2W], dt.float16, tag="dummy2")
                    nc.vector.memset(dummy2[:], BIG_NEG)
                    nc.sync.dma_start(
                        out=t2_shard[SH - npad:SH, 0:T2W], in_=dummy2[:npad, :])

                nc.gpsimd.collective_compute(
                    "AllGather", OP.bypass,
                    replica_groups=[list(range(CORES))],
                    ins=[t2_shard[:].opt()], outs=[t2c_full[:].opt()])
                for k in range(CORES):
                    nc.sync.dma_start(
                        out=t2_full[k * SH:(k + 1) * SH, 0:T2W],
                        in_=t2c_full[k * SH:(k + 1) * SH, :])

                # ---- phase C2: layer-2 aggregation + log_softmax ----
                for t in range(TILES):
                    SL = int(CT[t])
                    G2 = gpool.tile([P, SL * ROW], dt.float16, tag="G2")
                    gather_tile(G2, t, t2_full)
                    p2 = wpool.tile([P, SL], dt.float32, tag="p2")
                    nc.vector.tensor_scalar(
                        out=p2[:],
                        in0=G2[:].rearrange("p (d w) -> p d w", w=ROW)
                            [:, :, 0:1].squeeze(2),
                        scalar1=ald2[:, t:t + 1], scalar2=None, op0=OP.add)
                    nc.vector.scalar_tensor_tensor(
                        out=p2[:], in0=p2[:], scalar=NEG_SLOPE, in1=p2[:],
                        op0=OP.mult, op1=OP.max)
                    den2 = wpool.tile([P, 1], dt.float32, tag="den2")
                    nc.scalar.activation(p2[:], p2[:], AF.Exp, accum_out=den2[:])
                    rec2 = wpool.tile([P, 1], dt.float32, tag="rec2")
                    nc.vector.reciprocal(rec2[:], den2[:])
                    alpha2 = wpool.tile([P, SL], dt.float16, tag="alpha2")
                    nc.vector.tensor_scalar(
                        out=alpha2[:], in0=p2[:], scalar1=rec2[:, 0:1],
                        scalar2=None, op0=OP.mult)
                    G2p = wpool.tile([P, SL * NUM_CLASSES], dt.float16, tag="G2p")
                    G2_h = G2[:].rearrange("p (d w) -> p d w", w=ROW)[:, :, 1:T2W]
                    a2_b = alpha2[:].unsqueeze(2).broadcast_to(
                        [P, SL, NUM_CLASSES])
                    nc.vector.tensor_tensor(
                        out=G2p[:].rearrange("p (d w) -> p d w", w=NUM_CLASSES),
                        in0=G2_h, in1=a2_b, op=OP.mult)
                    o2 = wpool.tile([P, NUM_CLASSES], dt.float32, tag="o2")
                    nc.vector.tensor_reduce(
                        out=o2[:],
                        in_=G2p[:].rearrange("p (d w) -> p w d", w=NUM_CLASSES),
                        axis=AX.X, op=OP.add)
                    nc.vector.tensor_tensor(out=o2[:], in0=o2[:], in1=b2_t[:],
                                            op=OP.add)
                    # log_softmax over classes
                    mx = wpool.tile([P, 1], dt.float32, tag="mx")
                    nc.vector.tensor_reduce(out=mx[:], in_=o2[:], axis=AX.X,
                                            op=OP.max)
                    nc.vector.tensor_scalar(out=o2[:], in0=o2[:],
                                            scalar1=mx[:, 0:1],
                                            scalar2=None, op0=OP.subtract)
                    ex = wpool.tile([P, NUM_CLASSES], dt.float32, tag="ex")
                    sm = wpool.tile([P, 1], dt.float32, tag="sm")
                    nc.scalar.activation(ex[:], o2[:], AF.Exp, accum_out=sm[:])
                    lg = wpool.tile([P, 1], dt.float32, tag="lg")
                    nc.scalar.activation(lg[:], sm[:], AF.Ln)
                    nc.vector.tensor_scalar(out=o2[:], in0=o2[:],
                                            scalar1=lg[:, 0:1],
                                            scalar2=None, op0=OP.subtract)
                    nc.sync.dma_start(out=out[t * P:(t + 1) * P, :], in_=o2[:])
    nc.compile()
    return nc


def _make_runner(nc, n_cores=CORES):
    """Hold a jitted PJRT executable for repeated invocation."""
    import jax
    from jax.sharding import Mesh, PartitionSpec
    from jax.experimental.shard_map import shard_map
    from concourse import mybir
    from concourse.bass2jax import (_bass_exec_p, install_neuronx_cc_hook,
                                    partition_id_tensor)
    install_neuronx_cc_hook()
    partition_name = nc.partition_id_tensor.name if nc.partition_id_tensor else None
    in_names, out_names, out_avals, zero_outs = [], [], [], []
    for alloc in nc.m.functions[0].allocations:
        if not isinstance(alloc, mybir.MemoryLocationSet):
            continue
        name = alloc.memorylocations[0].name
        if alloc.kind == "ExternalInput":
            if name != partition_name:
                in_names.append(name)
        elif alloc.kind == "ExternalOutput":
            shape = tuple(alloc.tensor_shape)
            dtype = mybir.dt.np(alloc.dtype)
            out_names.append(name)
            out_avals.append(jax.core.ShapedArray(shape, dtype))
            zero_outs.append(np.zeros(shape, dtype))
    n_params = len(in_names)
    all_in = list(in_names) + list(out_names) + ([partition_name] if partition_name else [])

    def _body(*args):
        operands = list(args)
        if partition_name is not None:
            operands.append(partition_id_tensor())
        outs = _bass_exec_p.bind(
            *operands, out_avals=tuple(out_avals), in_names=tuple(all_in),
            out_names=tuple(out_names), lowering_input_output_aliases=(),
            sim_require_finite=True, sim_require_nnan=True, nc=nc)
        return tuple(outs)

    devices = jax.devices()[:n_cores]
    mesh = Mesh(np.asarray(devices), ("core",))
    nio = n_params + len(out_names)
    sharded = jax.jit(
        shard_map(_body, mesh=mesh, in_specs=(PartitionSpec("core"),) * nio,
                  out_specs=(PartitionSpec("core"),) * len(out_names),
                  check_rep=False),
        keep_unused=True)

    def run(in_maps, time_reps=0):
        import time as _t
        concat_in = [np.concatenate([np.asarray(in_maps[c][nm])
                                     for c in range(n_cores)], axis=0)
                     for nm in in_names]
        concat_zero = [np.zeros((n_cores * z.shape[0], *z.shape[1:]), z.dtype)
                       for z in zero_outs]
        dev_in = [jax.device_put(a) for a in concat_in]
        dev_zero = [jax.device_put(a) for a in concat_zero]
        outs = sharded(*dev_in, *dev_zero)
        jax.block_until_ready(outs)
        tmin = None
        if time_reps:
            ts = []
            for _ in range(time_reps):
                t0 = _t.perf_counter()
                outs = sharded(*dev_in, *dev_zero)
                jax.block_until_ready(outs)
                ts.append(_t.perf_counter() - t0)
            tmin = min(ts)
        results = [{nm: np.asarray(outs[i]).reshape(n_cores, *out_avals[i].shape)[c]
                    for i, nm in enumerate(out_names)} for c in range(n_cores)]
        return results, tmin

    return run


def _prepare_inputs(x, W1, a_src1, a_dst1, b1, W2, a_src2, a_dst2, b2, meta):
    SH = meta["shard_pad"]
    As = np.zeros((F_HID, HEADS), dtype=np.float32)
    Ad = np.zeros((F_HID, HEADS), dtype=np.float32)
    for h in range(HEADS):
        As[h * HID:(h + 1) * HID, h] = a_src1[h]
        Ad[h * HID:(h + 1) * HID, h] = a_dst1[h]
    w1aug = np.concatenate([W1 @ As, W1, W1 @ Ad], axis=1).astype(np.float16)
    w2aug = np.concatenate([W2 @ a_src2.T, W2, W2 @ a_dst2.T],
                           axis=1).astype(np.float16)
    b1rep = np.broadcast_to(b1[None, :], (P, F_HID)).copy()
    b2rep = np.broadcast_to(b2[None, :], (P, NUM_CLASSES)).copy()

    in_maps = []
    for r in range(CORES):
        perm = meta["perms"][r]
        xs = np.zeros((SH, F_IN), dtype=np.float16)
        xs[:len(perm)] = x[perm]
        in_maps.append({
            "xT": np.ascontiguousarray(xs.T),
            "w1aug": w1aug, "w2aug": w2aug,
            "b1rep": b1rep, "b2rep": b2rep,
            "idx": meta["idx"][r],
        })
    return in_maps


def kernel(x, edge_index, W1, a_src1, a_dst1, b1, W2, a_src2, a_dst2, b2,
           _time_reps=0, _repeat=1):
    x = np.asarray(x, dtype=np.float32)
    edge_index = np.asarray(edge_index)
    W1 = np.asarray(W1, dtype=np.float32)
    W2 = np.asarray(W2, dtype=np.float32)
    a_src1 = np.asarray(a_src1, dtype=np.float32)
    a_dst1 = np.asarray(a_dst1, dtype=np.float32)
    a_src2 = np.asarray(a_src2, dtype=np.float32)
    a_dst2 = np.asarray(a_dst2, dtype=np.float32)
    b1 = np.asarray(b1, dtype=np.float32)
    b2 = np.asarray(b2, dtype=np.float32)

    meta = _preprocess_graph(edge_index)
    key = (tuple(meta["CT"].tolist()), meta["shard_pad"], _repeat)
    if key not in _PROGRAM_CACHE:
        nc = _build_program(meta, repeat=_repeat)
        _PROGRAM_CACHE[key] = _make_runner(nc)
    run = _PROGRAM_CACHE[key]

    in_maps = _prepare_inputs(x, W1, a_src1, a_dst1, b1, W2, a_src2, a_dst2,
                              b2, meta)
    try:
        results, tmin = run(in_maps, time_reps=_time_reps)
    except Exception:
        # transient axon/NRT failures have been observed; one retry is cheap
        results, tmin = run(in_maps, time_reps=_time_reps)
    out = np.zeros((N, NUM_CLASSES), dtype=np.float32)
    for r in range(CORES):
        perm = meta["perms"][r]
        out[perm] = results[r]["out"][:len(perm)]
    if _time_reps:
        kernel._last_time_s = tmin
    return out
